# revision 17
# baseline (speedup 1.0000x reference)
"""Multi-head causal self-attention on 8 Trainium2 NeuronCores.

Problem: x [4, 2048, 1024], Wq/Wk/Wv/Wo [1024, 1024] (applied as x @ W.T),
16 heads, dk=64, causal softmax, output [4, 2048, 1024], all fp32.

Sharding: 8 cores = 4 batches x 2 head-groups (8 heads each).
Each core computes QKV projections for its 8 heads, streaming causal
attention, and a partial output projection (Wo row-split). The host adds
the two partial outputs per batch element.

Per-core layouts (chosen so NO on-device transposes are needed):
  xT  [1024, 2048]  = x[b].T          (host-transposed)
  wqT [1024, 512]   = (Wq/8).T cols for this head group (1/sqrt(dk) folded)
  wkT [1024, 512], wvT [1024, 512]
  woT [512, 1024]   = Wo[:, cols].T
  QT/KT on chip as [feat, seq] (head pairs stacked on partitions),
  V as [seq, feat] bf16. scoresT tiles [k=128, q=512] per head pair are
  exp'ed on ScalarE into bf16; the causal mask is applied with
  affine_select on the idle GpSimd engine; the softmax denominator is a
  ones-matmul (partition reduction on the PE); 1/l is broadcast across
  partitions with a tiny constant matmul.

Projection s-chunks and attention q-blocks are interleaved in program
order so TensorE (projections) and ScalarE (exp) work concurrently.
"""

import ml_dtypes
import numpy as np

import concourse.bass as bass
import concourse.mybir as mybir
import concourse.tile as tile
from concourse.bass_utils import run_bass_kernel_spmd
from concourse.vector_clock import ScopedClock

F32 = mybir.dt.float32
F32R = mybir.dt.float32r
BF16 = mybir.dt.bfloat16
AF = mybir.ActivationFunctionType
ALU = mybir.AluOpType

B, S, D = 4, 2048, 1024
H = 16
DK = 64
N_CORES = 8
HG = 512          # head-group width (8 heads x 64)


# ---------------------------------------------------------------------------
# This walrus accepts at most 1 sem wait per instruction (2 for
# EventSemaphore). Tile emits more in two places; both are fixed up here by
# moving excess waits onto preceding instructions on the same engine.
# ---------------------------------------------------------------------------
def _split_drain_and_barrier(self, tick_clock, wait_clock):
    nc = self.nc
    probe = nc.sync.nop(nofuse=True, hint="tile_drain_waits")
    wait_clock.add_sem_waits(
        probe.ins, ScopedClock({None: tick_clock.global_clock})
    )
    si = probe.ins.sync_info
    waits = list(si.on_wait) if si is not None else []
    if len(waits) > 1:
        probe.ins.sync_info = mybir.SyncInfo(on_wait=[waits[0]], on_update=[])
        for w in waits[1:]:
            n = nc.sync.nop(nofuse=True, hint="tile_drain_waits")
            n.ins.sync_info = mybir.SyncInfo(on_wait=[w], on_update=[])
    nc.sync.drain()
    nc.all_engine_barrier()
    popped = nc._tile_sem_poison_stack.pop()
    assert popped is self._sem_poison
    nc.clear_and_free_semaphores(list(self.sems.allocated().values()))
    nc.all_engine_barrier()


tile.TileContext._drain_and_barrier = _split_drain_and_barrier

_wsplit_counter = [0]


def _enforce_wait_limits(m):
    for fn in m.functions:
        for bb in fn.blocks:
            out = []
            changed = False
            for inst in bb.instructions:
                si = inst.sync_info
                cap = 2 if isinstance(inst, mybir.InstEventSemaphore) else 1
                if si is not None and len(si.on_wait) > cap:
                    waits = list(si.on_wait)
                    keep, extra = waits[:cap], waits[cap:]
                    for i in range(0, len(extra), 2):
                        _wsplit_counter[0] += 1
                        out.append(mybir.InstEventSemaphore(
                            name=f"I-wsplit-{_wsplit_counter[0]}",
                            engine=inst.engine,
                            ins=[], outs=[],
                            sync_info=mybir.SyncInfo(
                                on_wait=extra[i:i + 2], on_update=[]),
                        ))
                    inst.sync_info = mybir.SyncInfo(
                        on_wait=keep, on_update=list(si.on_update))
                    changed = True
                out.append(inst)
            if changed:
                bb.instructions = out


def build_nc():
    nc = bass.Bass()

    xT = nc.declare_dram_parameter("xT", [D, S], BF16, isOutput=False)
    wqT = nc.declare_dram_parameter("wqT", [D, HG], BF16, isOutput=False)
    wkT = nc.declare_dram_parameter("wkT", [D, HG], BF16, isOutput=False)
    wvT = nc.declare_dram_parameter("wvT", [D, HG], BF16, isOutput=False)
    woT = nc.declare_dram_parameter("woT", [HG, D], BF16, isOutput=False)
    bc33 = nc.declare_dram_parameter("bc33", [33, 128], F32R, isOutput=False)
    ones33 = nc.declare_dram_parameter("ones33", [33, 512], F32R, isOutput=False)
    yout = nc.declare_dram_parameter("y", [S, D], F32, isOutput=True)

    KT8 = D // 128   # contraction tiles for the projections
    NP = 4           # head pairs per core
    NS = S // 128    # seq tiles of 128

    from contextlib import ExitStack

    with tile.TileContext(nc) as tc, ExitStack() as ctx:
        ep = ctx.enter_context
        consts = ep(tc.tile_pool(name="consts", bufs=1))
        qt_pool = ep(tc.tile_pool(name="qt", bufs=1))
        kt_pool = ep(tc.tile_pool(name="kt", bufs=1))
        v_pool = ep(tc.tile_pool(name="v", bufs=1))
        wo_pool = ep(tc.tile_pool(name="wo", bufs=1))
        wq_pool = ep(tc.tile_pool(name="wq", bufs=1))
        wk_pool = ep(tc.tile_pool(name="wk", bufs=1))
        wv_pool = ep(tc.tile_pool(name="wv", bufs=1))
        xt_pool = ep(tc.tile_pool(name="xt", bufs=2))
        exp_pool = ep(tc.tile_pool(name="exp", bufs=4))
        ctxn_pool = ep(tc.tile_pool(name="ctxn", bufs=12))
        rcp_pool = ep(tc.tile_pool(name="rcp", bufs=2))
        ctxraw_pool = ep(tc.tile_pool(name="ctxraw", bufs=3))
        lpack_pool = ep(tc.tile_pool(name="lpack", bufs=2))
        dram_pool = ep(tc.tile_pool(name="ldram", bufs=2, space="DRAM"))
        lraw_pool = ep(tc.tile_pool(name="lraw", bufs=3))
        bcs_pool = ep(tc.tile_pool(name="bcs", bufs=3))
        ybuf_pool = ep(tc.tile_pool(name="ybuf", bufs=2))
        mm_ps = ep(tc.tile_pool(name="mm_ps", bufs=2, space="PSUM"))
        sc_ps = ep(tc.tile_pool(name="sc_ps", bufs=2, space="PSUM"))
        ctx_ps = ep(tc.tile_pool(name="ctx_ps", bufs=1, space="PSUM"))
        l_ps = ep(tc.tile_pool(name="l_ps", bufs=1, space="PSUM"))

        # ---- constants and weights ----------------------------------------
        bc_t = consts.tile([33, 128], F32R, tag="bc")
        nc.sync.dma_start(bc_t[:], bc33[:])
        ones_t = consts.tile([128, 1], BF16, tag="ones")
        nc.gpsimd.memset(ones_t[:], 1.0)

        QT = [qt_pool.tile([128, S], BF16, tag=f"qt{p}", name=f"QT{p}")
              for p in range(NP)]
        KTt = [kt_pool.tile([128, S], BF16, tag=f"kt{p}", name=f"KTt{p}")
               for p in range(NP)]
        V = [v_pool.tile([128, HG], BF16, tag=f"v{s}", name=f"V{s}")
             for s in range(NS)]
        wo_t = []
        for c in range(NP):
            t = wo_pool.tile([128, D], BF16, tag=f"wo{c}")
            nc.sync.dma_start(t[:], woT[c * 128:(c + 1) * 128, :])
            wo_t.append(t)
        wq_t, wk_t, wv_t = [], [], []
        for kt in range(KT8):
            for pool, lst, src in (
                (wq_pool, wq_t, wqT),
                (wk_pool, wk_t, wkT),
                (wv_pool, wv_t, wvT),
            ):
                t = pool.tile([128, HG], BF16, tag=f"w{kt}")
                nc.sync.dma_start(t[:], src[kt * 128:(kt + 1) * 128, :])
                lst.append(t)

        def emit_xt_dmas(st):
            xts = []
            for kt in range(KT8):
                t = xt_pool.tile([128, 512], BF16, tag=f"xt{kt}",
                                 name=f"xt{st}_{kt}")
                nc.sync.dma_start(
                    t[:], xT[kt * 128:(kt + 1) * 128, st * 512:(st + 1) * 512]
                )
                xts.append(t)
            return xts

        def proj_items(st, xts):
            """QKV projection work for chunk st as a flat list of closures,
            one instruction each, so they can be sprinkled between attention
            triples at fine grain."""
            items = []

            def qk_group(ot, w_t, dst, name):
                holder = {}

                def mk_mm(kt):
                    def go():
                        if "ps" not in holder:
                            holder["ps"] = mm_ps.tile(
                                [128, 512], F32, tag="mm", name=name)
                        nc.tensor.matmul(
                            holder["ps"][:],
                            w_t[kt][:, ot * 128:(ot + 1) * 128],
                            xts[kt][:],
                            start=(kt == 0),
                            stop=(kt == KT8 - 1),
                        )
                    return go

                def copy():
                    nc.vector.tensor_copy(
                        dst[ot][:, st * 512:(st + 1) * 512], holder["ps"][:])

                return [mk_mm(kt) for kt in range(KT8)] + [copy]

            def v_group(sub):
                holder = {}

                def mk_mm(kt):
                    def go():
                        if "ps" not in holder:
                            holder["ps"] = mm_ps.tile(
                                [128, 512], F32, tag="mm", name=f"pv{st}{sub}")
                        nc.tensor.matmul(
                            holder["ps"][:],
                            xts[kt][:, sub * 128:(sub + 1) * 128],
                            wv_t[kt][:],
                            start=(kt == 0),
                            stop=(kt == KT8 - 1),
                        )
                    return go

                def copy():
                    nc.vector.tensor_copy(V[st * 4 + sub][:], holder["ps"][:])

                return [mk_mm(kt) for kt in range(KT8)] + [copy]

            for ot in range(NP):
                items.extend(qk_group(ot, wq_t, QT, f"pq{st}{ot}"))
                items.extend(qk_group(ot, wk_t, KTt, f"pk{st}{ot}"))
            for sub in range(4):
                items.extend(v_group(sub))
            return items

        def attention_block(j, fill):
            """Causal attention + partial output projection for q-tile j.
            `fill` is a list of closures (next chunk's projection groups)
            sprinkled into the PE stream to cover exp-wait stalls."""
            fill = list(fill)
            n_triples = NP * 4 * (j + 1)
            per_triple = -(-len(fill) // n_triples) if fill else 0

            def emit_fill(n):
                for _ in range(n):
                    if fill:
                        fill.pop(0)()

            def scores(pair, j, i):
                sc = sc_ps.tile([128, 1024], F32, tag="sc",
                                name=f"sc{j}{pair}{i}")
                qa = QT[pair][0:64, j * 512:(j + 1) * 512]
                qb = QT[pair][64:128, j * 512:(j + 1) * 512]
                ka = KTt[pair][0:64, i * 128:(i + 1) * 128]
                kb = KTt[pair][64:128, i * 128:(i + 1) * 128]
                nc.tensor.matmul(
                    sc[:, 0:512], ka, qa,
                    start=True, stop=True, tile_position=(0, 0),
                )
                nc.tensor.matmul(
                    sc[:, 512:1024], kb, qb,
                    start=True, stop=True, tile_position=(64, 0),
                )
                return sc

            ctxn = []
            ni = 4 * (j + 1)

            def emit_ctx_l(ctx_t, lps, et, i):
                first, last = (i == 0), (i == ni - 1)
                va = V[i][:, pair * 128:pair * 128 + 64]
                vb = V[i][:, pair * 128 + 64:pair * 128 + 128]
                nc.tensor.matmul(
                    ctx_t[0:64, :], va, et[:, 0:512],
                    start=first, stop=last, tile_position=(0, 0),
                )
                nc.tensor.matmul(
                    ctx_t[64:128, :], vb, et[:, 512:1024],
                    start=first, stop=last, tile_position=(0, 64),
                )
                nc.tensor.matmul(
                    lps[0:1, :], ones_t[:], et[:, 0:512],
                    start=first, stop=last, tile_position=(0, 0),
                )
                nc.tensor.matmul(
                    lps[32:33, :], ones_t[:], et[:, 512:1024],
                    start=first, stop=last, tile_position=(0, 32),
                )

            def emit_normalize(ctxraw, lraw, pair):
                # ctx[c, q] /= l[q]: broadcast 1/l across partitions with a
                # tiny constant matmul.
                # DVE reciprocal is per-lane-serial: a [1,512] recip costs
                # ~3.3us on one lane. Repack l across 128 partitions via a
                # DRAM bounce so the recip runs on all lanes (~0.1us).
                ls = dram_pool.tile([2, 512], F32, tag="ls",
                                    name=f"ls{j}{pair}")
                nc.sync.dma_start(ls[0:1, :], lraw[0:1, :])
                nc.sync.dma_start(ls[1:2, :], lraw[32:33, :])
                lpack = lpack_pool.tile([128, 8], F32, tag="lp",
                                        name=f"lp{j}{pair}")
                nc.sync.dma_start(
                    lpack[:, 0:4], ls[0].rearrange("(p f) -> p f", p=128))
                nc.sync.dma_start(
                    lpack[:, 4:8], ls[1].rearrange("(p f) -> p f", p=128))
                rpk = lpack_pool.tile([128, 8], F32R, tag="rp",
                                      name=f"rp{j}{pair}")
                with nc.allow_low_precision("fp32r attention pipeline"):
                    nc.vector.reciprocal(rpk[:], lpack[:])
                rs = dram_pool.tile([2, 512], F32R, tag="rs",
                                    name=f"rs{j}{pair}")
                nc.sync.dma_start(
                    rs[0].rearrange("(p f) -> p f", p=128), rpk[:, 0:4])
                nc.sync.dma_start(
                    rs[1].rearrange("(p f) -> p f", p=128), rpk[:, 4:8])
                rcp = rcp_pool.tile([33, 512], F32R, tag="rcp",
                                    name=f"rcp{j}{pair}")
                nc.sync.dma_start(rcp[:], ones33[:])
                nc.sync.dma_start(rcp[0:1, :], rs[0:1, :])
                nc.sync.dma_start(rcp[32:33, :], rs[1:2, :])
                bcp = mm_ps.tile([128, 512], F32, tag="mm", name=f"bcp{j}{pair}")
                nc.tensor.matmul(bcp[:], bc_t[:], rcp[:], start=True, stop=True)
                bcs = bcs_pool.tile([128, 512], F32, tag="bcs",
                                    name=f"bcs{j}{pair}")
                nc.vector.tensor_copy(bcs[:], bcp[:])
                cn = ctxn_pool.tile([128, 512], BF16, tag="cn",
                                    name=f"cn{j}{pair}")
                nc.vector.tensor_mul(cn[:], ctxraw[:], bcs[:])
                ctxn.append(cn)

            pending_norm = None
            for pair in range(NP):
                ctx_t = ctx_ps.tile([128, 512], F32, tag="ctx",
                                    name=f"ctx{j}{pair}")
                lps = l_ps.tile([33, 512], F32, tag="l", name=f"l{j}{pair}")
                sc = scores(pair, j, 0)
                pending = None
                for i in range(ni):
                    et = exp_pool.tile([128, 1024], BF16, tag="exp",
                                       name=f"et{j}{pair}{i}")
                    nc.scalar.activation(et[:], sc[:], AF.Exp)
                    if i >= 4 * j:
                        # diagonal block: zero the future positions
                        # keep et[kk, h, qq] iff qq - kk - 128*(i-4j) >= 0
                        p = i - 4 * j
                        nc.gpsimd.affine_select(
                            out=et[:], in_=et[:],
                            pattern=[[0, 2], [1, 512]],
                            compare_op=ALU.is_ge,
                            fill=0.0,
                            base=-128 * p,
                            channel_multiplier=-1,
                        )
                    if i + 1 < ni:
                        sc = scores(pair, j, i + 1)
                    if pending is not None:
                        emit_ctx_l(ctx_t, lps, *pending)
                        emit_fill(per_triple)
                    pending = (et, i)
                emit_ctx_l(ctx_t, lps, *pending)
                emit_fill(per_triple)
                # drain ctx/l PSUM banks to SBUF with single DVE copies so the
                # next pair's matmuls aren't gated on the normalize chain
                ctxraw = ctxraw_pool.tile([128, 512], F32, tag="cr",
                                          name=f"cr{j}{pair}")
                nc.vector.tensor_copy(ctxraw[:], ctx_t[:])
                lraw = lraw_pool.tile([33, 512], F32, tag="lr",
                                      name=f"lr{j}{pair}")
                nc.vector.tensor_copy(lraw[:], lps[:])
                if pending_norm is not None:
                    emit_normalize(*pending_norm)
                pending_norm = (ctxraw, lraw, pair)
            emit_normalize(*pending_norm)

            return ctxn

        def outproj_items(j, ctxn):
            """Output projection for q-tile j as fine-grain fill items."""
            items = []

            def group(s4, oh, holder):
                def mk_mm(pair):
                    def go():
                        if "ps" not in holder:
                            holder["ps"] = mm_ps.tile(
                                [128, 512], F32, tag="mm", name=f"yp{j}{s4}{oh}")
                        nc.tensor.matmul(
                            holder["ps"][:],
                            ctxn[pair][:, s4 * 128:(s4 + 1) * 128],
                            wo_t[pair][:, oh * 512:(oh + 1) * 512],
                            start=(pair == 0),
                            stop=(pair == NP - 1),
                        )
                    return go

                def copy():
                    nc.vector.tensor_copy(
                        holder["yb"][:, oh * 512:(oh + 1) * 512], holder["ps"][:])
                    del holder["ps"]

                return [mk_mm(p) for p in range(NP)] + [copy]

            for s4 in range(4):
                srow = j * 4 + s4
                holder = {}

                def alloc_yb(holder=holder, s4=s4):
                    holder["yb"] = ybuf_pool.tile(
                        [128, D], F32, tag="yb", name=f"yb{j}{s4}")

                items.append(alloc_yb)
                for oh in range(2):
                    items.extend(group(s4, oh, holder))

                def dma_out(holder=holder, srow=srow):
                    nc.sync.dma_start(
                        yout[srow * 128:(srow + 1) * 128, :], holder["yb"][:])

                items.append(dma_out)
            return items

        # chunk 0 projections run alone; attention block j then carries
        # chunk j+1's projections and block j-1's output projection as PE
        # filler for its exp-wait stalls.
        xts0 = emit_xt_dmas(0)
        for item in proj_items(0, xts0):
            item()
        prev_out = []
        for j in range(4):
            fill = list(prev_out)
            if j + 1 < 4:
                xts = emit_xt_dmas(j + 1)
                fill = proj_items(j + 1, xts) + fill
            ctxn = attention_block(j, fill)
            prev_out = outproj_items(j, ctxn)
        for item in prev_out:
            item()

    _enforce_wait_limits(nc.m)
    return nc


def _host_constants():
    bc = np.zeros((33, 128), np.float32)
    bc[0, 0:64] = 1.0
    bc[32, 64:128] = 1.0
    ones33 = np.ones((33, 512), np.float32)
    return bc, ones33


_NC = None


def _get_nc():
    global _NC
    if _NC is None:
        _NC = build_nc()
    return _NC


def run(x, Wq, Wk, Wv, Wo, trace=False, trace_kwargs=None):
    """Returns (y, BassKernelResults)."""
    x = np.asarray(x, np.float32)
    bc, ones33 = _host_constants()
    scale = 1.0 / np.sqrt(DK)
    in_maps = []
    for core in range(N_CORES):
        b, g = core // 2, core % 2
        cols = slice(g * HG, (g + 1) * HG)
        bf = ml_dtypes.bfloat16
        in_maps.append({
            "xT": np.ascontiguousarray(x[b].T).astype(bf),
            "wqT": np.ascontiguousarray(
                np.asarray(Wq, np.float32).T[:, cols] * scale).astype(bf),
            "wkT": np.ascontiguousarray(
                np.asarray(Wk, np.float32).T[:, cols]).astype(bf),
            "wvT": np.ascontiguousarray(
                np.asarray(Wv, np.float32).T[:, cols]).astype(bf),
            "woT": np.ascontiguousarray(
                np.asarray(Wo, np.float32).T[cols, :]).astype(bf),
            "bc33": bc,
            "ones33": ones33,
        })
    kw = dict(trace_kwargs or {})
    res = run_bass_kernel_spmd(
        _get_nc(), in_maps, list(range(N_CORES)), trace=trace, **kw
    )
    y = np.empty((B, S, D), np.float32)
    for b in range(B):
        y[b] = res.results[2 * b]["y"] + res.results[2 * b + 1]["y"]
    return y, res


def kernel(x, Wq, Wk, Wv, Wo):
    y, _ = run(x, Wq, Wk, Wv, Wo)
    return y


# revision 18
# speedup vs baseline: 1.0134x; 1.0134x over previous
"""Multi-head causal self-attention on 8 Trainium2 NeuronCores.

Problem: x [4, 2048, 1024], Wq/Wk/Wv/Wo [1024, 1024] (applied as x @ W.T),
16 heads, dk=64, causal softmax, output [4, 2048, 1024], all fp32.

Sharding: 8 cores = 4 batches x 2 head-groups (8 heads each).
Each core computes QKV projections for its 8 heads, streaming causal
attention, and a partial output projection (Wo row-split). The host adds
the two partial outputs per batch element.

Per-core layouts (chosen so NO on-device transposes are needed):
  xT  [1024, 2048]  = x[b].T          (host-transposed)
  wqT [1024, 512]   = (Wq/8).T cols for this head group (1/sqrt(dk) folded)
  wkT [1024, 512], wvT [1024, 512]
  woT [512, 1024]   = Wo[:, cols].T
  QT/KT on chip as [feat, seq] (head pairs stacked on partitions),
  V as [seq, feat] bf16. scoresT tiles [k=128, q=512] per head pair are
  exp'ed on ScalarE into bf16; the causal mask is applied with
  affine_select on the idle GpSimd engine; the softmax denominator is a
  ones-matmul (partition reduction on the PE); 1/l is broadcast across
  partitions with a tiny constant matmul.

Projection s-chunks and attention q-blocks are interleaved in program
order so TensorE (projections) and ScalarE (exp) work concurrently.
"""

import ml_dtypes
import numpy as np

import concourse.bass as bass
import concourse.mybir as mybir
import concourse.tile as tile
from concourse.bass_utils import run_bass_kernel_spmd
from concourse.vector_clock import ScopedClock

F32 = mybir.dt.float32
F32R = mybir.dt.float32r
BF16 = mybir.dt.bfloat16
AF = mybir.ActivationFunctionType
ALU = mybir.AluOpType

B, S, D = 4, 2048, 1024
H = 16
DK = 64
N_CORES = 8
HG = 512          # head-group width (8 heads x 64)


# ---------------------------------------------------------------------------
# This walrus accepts at most 1 sem wait per instruction (2 for
# EventSemaphore). Tile emits more in two places; both are fixed up here by
# moving excess waits onto preceding instructions on the same engine.
# ---------------------------------------------------------------------------
def _split_drain_and_barrier(self, tick_clock, wait_clock):
    nc = self.nc
    probe = nc.sync.nop(nofuse=True, hint="tile_drain_waits")
    wait_clock.add_sem_waits(
        probe.ins, ScopedClock({None: tick_clock.global_clock})
    )
    si = probe.ins.sync_info
    waits = list(si.on_wait) if si is not None else []
    if len(waits) > 1:
        probe.ins.sync_info = mybir.SyncInfo(on_wait=[waits[0]], on_update=[])
        for w in waits[1:]:
            n = nc.sync.nop(nofuse=True, hint="tile_drain_waits")
            n.ins.sync_info = mybir.SyncInfo(on_wait=[w], on_update=[])
    nc.sync.drain()
    nc.all_engine_barrier()
    popped = nc._tile_sem_poison_stack.pop()
    assert popped is self._sem_poison
    nc.clear_and_free_semaphores(list(self.sems.allocated().values()))
    nc.all_engine_barrier()


tile.TileContext._drain_and_barrier = _split_drain_and_barrier

_wsplit_counter = [0]


def _enforce_wait_limits(m):
    for fn in m.functions:
        for bb in fn.blocks:
            out = []
            changed = False
            for inst in bb.instructions:
                si = inst.sync_info
                cap = 2 if isinstance(inst, mybir.InstEventSemaphore) else 1
                if si is not None and len(si.on_wait) > cap:
                    waits = list(si.on_wait)
                    keep, extra = waits[:cap], waits[cap:]
                    for i in range(0, len(extra), 2):
                        _wsplit_counter[0] += 1
                        out.append(mybir.InstEventSemaphore(
                            name=f"I-wsplit-{_wsplit_counter[0]}",
                            engine=inst.engine,
                            ins=[], outs=[],
                            sync_info=mybir.SyncInfo(
                                on_wait=extra[i:i + 2], on_update=[]),
                        ))
                    inst.sync_info = mybir.SyncInfo(
                        on_wait=keep, on_update=list(si.on_update))
                    changed = True
                out.append(inst)
            if changed:
                bb.instructions = out


def build_nc():
    nc = bass.Bass()

    xT = nc.declare_dram_parameter("xT", [D, S], BF16, isOutput=False)
    wqT = nc.declare_dram_parameter("wqT", [D, HG], BF16, isOutput=False)
    wkT = nc.declare_dram_parameter("wkT", [D, HG], BF16, isOutput=False)
    wvT = nc.declare_dram_parameter("wvT", [D, HG], BF16, isOutput=False)
    woT = nc.declare_dram_parameter("woT", [HG, D], BF16, isOutput=False)
    bc33 = nc.declare_dram_parameter("bc33", [33, 128], F32R, isOutput=False)
    ones33 = nc.declare_dram_parameter("ones33", [33, 512], F32R, isOutput=False)
    yout = nc.declare_dram_parameter("y", [S, D], F32, isOutput=True)

    KT8 = D // 128   # contraction tiles for the projections
    NP = 4           # head pairs per core
    NS = S // 128    # seq tiles of 128

    from contextlib import ExitStack

    with tile.TileContext(nc) as tc, ExitStack() as ctx:
        ep = ctx.enter_context
        consts = ep(tc.tile_pool(name="consts", bufs=1))
        qt_pool = ep(tc.tile_pool(name="qt", bufs=1))
        kt_pool = ep(tc.tile_pool(name="kt", bufs=1))
        v_pool = ep(tc.tile_pool(name="v", bufs=1))
        wo_pool = ep(tc.tile_pool(name="wo", bufs=1))
        wq_pool = ep(tc.tile_pool(name="wq", bufs=1))
        wk_pool = ep(tc.tile_pool(name="wk", bufs=1))
        wv_pool = ep(tc.tile_pool(name="wv", bufs=1))
        xt_pool = ep(tc.tile_pool(name="xt", bufs=2))
        exp_pool = ep(tc.tile_pool(name="exp", bufs=3))
        ctxn_pool = ep(tc.tile_pool(name="ctxn", bufs=12))
        rcp_pool = ep(tc.tile_pool(name="rcp", bufs=2))
        ctxraw_pool = ep(tc.tile_pool(name="ctxraw", bufs=2))
        lpack_pool = ep(tc.tile_pool(name="lpack", bufs=2))
        dram_pool = ep(tc.tile_pool(name="ldram", bufs=2, space="DRAM"))
        lraw_pool = ep(tc.tile_pool(name="lraw", bufs=2))
        bcs_pool = ep(tc.tile_pool(name="bcs", bufs=2))
        ybuf_pool = ep(tc.tile_pool(name="ybuf", bufs=2))
        mm_ps = ep(tc.tile_pool(name="mm_ps", bufs=2, space="PSUM"))
        sc_ps = ep(tc.tile_pool(name="sc_ps", bufs=2, space="PSUM"))
        ctx_ps = ep(tc.tile_pool(name="ctx_ps", bufs=1, space="PSUM"))
        l_ps = ep(tc.tile_pool(name="l_ps", bufs=1, space="PSUM"))

        # ---- constants and weights ----------------------------------------
        bc_t = consts.tile([33, 128], F32R, tag="bc")
        nc.sync.dma_start(bc_t[:], bc33[:])
        ones_t = consts.tile([128, 1], BF16, tag="ones")
        nc.gpsimd.memset(ones_t[:], 1.0)

        QT = [qt_pool.tile([128, S], BF16, tag=f"qt{p}", name=f"QT{p}")
              for p in range(NP)]
        KTt = [kt_pool.tile([128, S], BF16, tag=f"kt{p}", name=f"KTt{p}")
               for p in range(NP)]
        V = [v_pool.tile([128, HG], BF16, tag=f"v{s}", name=f"V{s}")
             for s in range(NS)]
        wo_t = []
        for c in range(NP):
            t = wo_pool.tile([128, D], BF16, tag=f"wo{c}")
            nc.sync.dma_start(t[:], woT[c * 128:(c + 1) * 128, :])
            wo_t.append(t)
        wq_t, wk_t, wv_t = [], [], []
        for kt in range(KT8):
            for pool, lst, src in (
                (wq_pool, wq_t, wqT),
                (wk_pool, wk_t, wkT),
                (wv_pool, wv_t, wvT),
            ):
                t = pool.tile([128, HG], BF16, tag=f"w{kt}")
                nc.sync.dma_start(t[:], src[kt * 128:(kt + 1) * 128, :])
                lst.append(t)

        def emit_xt_dmas(st):
            xts = []
            for kt in range(KT8):
                t = xt_pool.tile([128, 512], BF16, tag=f"xt{kt}",
                                 name=f"xt{st}_{kt}")
                nc.sync.dma_start(
                    t[:], xT[kt * 128:(kt + 1) * 128, st * 512:(st + 1) * 512]
                )
                xts.append(t)
            return xts

        def proj_items(st, xts):
            """QKV projection work for chunk st as a flat list of closures,
            one instruction each, so they can be sprinkled between attention
            triples at fine grain."""
            items = []

            def qk_group(ot, w_t, dst, name):
                holder = {}

                def mk_mm(kt):
                    def go():
                        if "ps" not in holder:
                            holder["ps"] = mm_ps.tile(
                                [128, 512], F32, tag="mm", name=name)
                        nc.tensor.matmul(
                            holder["ps"][:],
                            w_t[kt][:, ot * 128:(ot + 1) * 128],
                            xts[kt][:],
                            start=(kt == 0),
                            stop=(kt == KT8 - 1),
                        )
                    return go

                def copy():
                    nc.vector.tensor_copy(
                        dst[ot][:, st * 512:(st + 1) * 512], holder["ps"][:])

                return [mk_mm(kt) for kt in range(KT8)] + [copy]

            def v_group(sub):
                holder = {}

                def mk_mm(kt):
                    def go():
                        if "ps" not in holder:
                            holder["ps"] = mm_ps.tile(
                                [128, 512], F32, tag="mm", name=f"pv{st}{sub}")
                        nc.tensor.matmul(
                            holder["ps"][:],
                            xts[kt][:, sub * 128:(sub + 1) * 128],
                            wv_t[kt][:],
                            start=(kt == 0),
                            stop=(kt == KT8 - 1),
                        )
                    return go

                def copy():
                    nc.vector.tensor_copy(V[st * 4 + sub][:], holder["ps"][:])

                return [mk_mm(kt) for kt in range(KT8)] + [copy]

            for ot in range(NP):
                items.extend(qk_group(ot, wq_t, QT, f"pq{st}{ot}"))
                items.extend(qk_group(ot, wk_t, KTt, f"pk{st}{ot}"))
            for sub in range(4):
                items.extend(v_group(sub))
            return items

        def attention_block(j, fill):
            """Causal attention + partial output projection for q-tile j.
            `fill` is a list of closures (next chunk's projection groups)
            sprinkled into the PE stream to cover exp-wait stalls."""
            fill = list(fill)
            n_triples = NP * 4 * (j + 1)
            per_triple = -(-len(fill) // n_triples) if fill else 0

            def emit_fill(n):
                for _ in range(n):
                    if fill:
                        fill.pop(0)()

            def scores(pair, j, i):
                sc = sc_ps.tile([128, 1024], F32, tag="sc",
                                name=f"sc{j}{pair}{i}")
                qa = QT[pair][0:64, j * 512:(j + 1) * 512]
                qb = QT[pair][64:128, j * 512:(j + 1) * 512]
                ka = KTt[pair][0:64, i * 128:(i + 1) * 128]
                kb = KTt[pair][64:128, i * 128:(i + 1) * 128]
                nc.tensor.matmul(
                    sc[:, 0:512], ka, qa,
                    start=True, stop=True, tile_position=(0, 0),
                )
                nc.tensor.matmul(
                    sc[:, 512:1024], kb, qb,
                    start=True, stop=True, tile_position=(64, 0),
                )
                return sc

            ctxn = []
            ni = 4 * (j + 1)

            def emit_ctx_l(ctx_t, lps, et, i):
                first, last = (i == 0), (i == ni - 1)
                va = V[i][:, pair * 128:pair * 128 + 64]
                vb = V[i][:, pair * 128 + 64:pair * 128 + 128]
                nc.tensor.matmul(
                    ctx_t[0:64, :], va, et[:, 0:512],
                    start=first, stop=last, tile_position=(0, 0),
                )
                nc.tensor.matmul(
                    ctx_t[64:128, :], vb, et[:, 512:1024],
                    start=first, stop=last, tile_position=(0, 64),
                )
                nc.tensor.matmul(
                    lps[0:1, :], ones_t[:], et[:, 0:512],
                    start=first, stop=last, tile_position=(0, 0),
                )
                nc.tensor.matmul(
                    lps[32:33, :], ones_t[:], et[:, 512:1024],
                    start=first, stop=last, tile_position=(0, 32),
                )

            def emit_normalize(ctxraw, lraw, pair):
                # ctx[c, q] /= l[q]: broadcast 1/l across partitions with a
                # tiny constant matmul.
                # DVE reciprocal is per-lane-serial: a [1,512] recip costs
                # ~3.3us on one lane. Repack l across 128 partitions via a
                # DRAM bounce so the recip runs on all lanes (~0.1us).
                ls = dram_pool.tile([2, 512], F32, tag="ls",
                                    name=f"ls{j}{pair}")
                nc.sync.dma_start(ls[0:1, :], lraw[0:1, :])
                nc.sync.dma_start(ls[1:2, :], lraw[32:33, :])
                lpack = lpack_pool.tile([128, 8], F32, tag="lp",
                                        name=f"lp{j}{pair}")
                nc.sync.dma_start(
                    lpack[:, 0:4], ls[0].rearrange("(p f) -> p f", p=128))
                nc.sync.dma_start(
                    lpack[:, 4:8], ls[1].rearrange("(p f) -> p f", p=128))
                rpk = lpack_pool.tile([128, 8], F32R, tag="rp",
                                      name=f"rp{j}{pair}")
                with nc.allow_low_precision("fp32r attention pipeline"):
                    nc.vector.reciprocal(rpk[:], lpack[:])
                rs = dram_pool.tile([2, 512], F32R, tag="rs",
                                    name=f"rs{j}{pair}")
                nc.sync.dma_start(
                    rs[0].rearrange("(p f) -> p f", p=128), rpk[:, 0:4])
                nc.sync.dma_start(
                    rs[1].rearrange("(p f) -> p f", p=128), rpk[:, 4:8])
                rcp = rcp_pool.tile([33, 512], F32R, tag="rcp",
                                    name=f"rcp{j}{pair}")
                nc.sync.dma_start(rcp[:], ones33[:])
                nc.sync.dma_start(rcp[0:1, :], rs[0:1, :])
                nc.sync.dma_start(rcp[32:33, :], rs[1:2, :])
                bcp = mm_ps.tile([128, 512], F32, tag="mm", name=f"bcp{j}{pair}")
                nc.tensor.matmul(bcp[:], bc_t[:], rcp[:], start=True, stop=True)
                bcs = bcs_pool.tile([128, 512], F32, tag="bcs",
                                    name=f"bcs{j}{pair}")
                nc.vector.tensor_copy(bcs[:], bcp[:])
                cn = ctxn_pool.tile([128, 512], BF16, tag="cn",
                                    name=f"cn{j}{pair}")
                nc.vector.tensor_mul(cn[:], ctxraw[:], bcs[:])
                ctxn.append(cn)

            pending_norm = None
            for pair in range(NP):
                ctx_t = ctx_ps.tile([128, 512], F32, tag="ctx",
                                    name=f"ctx{j}{pair}")
                lps = l_ps.tile([33, 512], F32, tag="l", name=f"l{j}{pair}")
                sc = scores(pair, j, 0)
                pending = None
                for i in range(ni):
                    et = exp_pool.tile([128, 1024], BF16, tag="exp",
                                       name=f"et{j}{pair}{i}")
                    nc.scalar.activation(et[:], sc[:], AF.Exp)
                    if i >= 4 * j:
                        # diagonal block: zero the future positions
                        # keep et[kk, h, qq] iff qq - kk - 128*(i-4j) >= 0
                        p = i - 4 * j
                        nc.gpsimd.affine_select(
                            out=et[:], in_=et[:],
                            pattern=[[0, 2], [1, 512]],
                            compare_op=ALU.is_ge,
                            fill=0.0,
                            base=-128 * p,
                            channel_multiplier=-1,
                        )
                    if i + 1 < ni:
                        sc = scores(pair, j, i + 1)
                    if pending is not None:
                        emit_ctx_l(ctx_t, lps, *pending)
                        emit_fill(per_triple)
                    pending = (et, i)
                emit_ctx_l(ctx_t, lps, *pending)
                emit_fill(per_triple)
                # drain ctx/l PSUM banks to SBUF with single DVE copies so the
                # next pair's matmuls aren't gated on the normalize chain
                ctxraw = ctxraw_pool.tile([128, 512], F32, tag="cr",
                                          name=f"cr{j}{pair}")
                nc.vector.tensor_copy(ctxraw[:], ctx_t[:])
                lraw = lraw_pool.tile([33, 512], F32, tag="lr",
                                      name=f"lr{j}{pair}")
                nc.vector.tensor_copy(lraw[:], lps[:])
                if pending_norm is not None:
                    emit_normalize(*pending_norm)
                pending_norm = (ctxraw, lraw, pair)
            emit_normalize(*pending_norm)

            return ctxn

        def outproj_items(j, ctxn):
            """Output projection for q-tile j as fine-grain fill items."""
            items = []

            def group(s4, oh, holder):
                def mk_mm(pair):
                    def go():
                        if "ps" not in holder:
                            holder["ps"] = mm_ps.tile(
                                [128, 512], F32, tag="mm", name=f"yp{j}{s4}{oh}")
                        nc.tensor.matmul(
                            holder["ps"][:],
                            ctxn[pair][:, s4 * 128:(s4 + 1) * 128],
                            wo_t[pair][:, oh * 512:(oh + 1) * 512],
                            start=(pair == 0),
                            stop=(pair == NP - 1),
                        )
                    return go

                def copy():
                    nc.vector.tensor_copy(
                        holder["yb"][:, oh * 512:(oh + 1) * 512], holder["ps"][:])
                    del holder["ps"]

                return [mk_mm(p) for p in range(NP)] + [copy]

            for s4 in range(4):
                srow = j * 4 + s4
                holder = {}

                def alloc_yb(holder=holder, s4=s4):
                    holder["yb"] = ybuf_pool.tile(
                        [128, D], F32, tag="yb", name=f"yb{j}{s4}")

                items.append(alloc_yb)
                for oh in range(2):
                    items.extend(group(s4, oh, holder))

                def dma_out(holder=holder, srow=srow):
                    nc.sync.dma_start(
                        yout[srow * 128:(srow + 1) * 128, :], holder["yb"][:])

                items.append(dma_out)
            return items

        # chunk 0 projections run alone; attention block j then carries
        # chunk j+1's projections and block j-1's output projection as PE
        # filler for its exp-wait stalls.
        xts0 = emit_xt_dmas(0)
        for item in proj_items(0, xts0):
            item()
        prev_out = []
        for j in range(4):
            fill = list(prev_out)
            if j + 1 < 4:
                xts = emit_xt_dmas(j + 1)
                fill = proj_items(j + 1, xts) + fill
            ctxn = attention_block(j, fill)
            prev_out = outproj_items(j, ctxn)
        for item in prev_out:
            item()

    _enforce_wait_limits(nc.m)
    return nc


def _host_constants():
    bc = np.zeros((33, 128), np.float32)
    bc[0, 0:64] = 1.0
    bc[32, 64:128] = 1.0
    ones33 = np.ones((33, 512), np.float32)
    return bc, ones33


_NC = None


def _get_nc():
    global _NC
    if _NC is None:
        _NC = build_nc()
    return _NC


def run(x, Wq, Wk, Wv, Wo, trace=False, trace_kwargs=None):
    """Returns (y, BassKernelResults)."""
    x = np.asarray(x, np.float32)
    bc, ones33 = _host_constants()
    scale = 1.0 / np.sqrt(DK)
    in_maps = []
    for core in range(N_CORES):
        b, g = core // 2, core % 2
        cols = slice(g * HG, (g + 1) * HG)
        bf = ml_dtypes.bfloat16
        in_maps.append({
            "xT": np.ascontiguousarray(x[b].T).astype(bf),
            "wqT": np.ascontiguousarray(
                np.asarray(Wq, np.float32).T[:, cols] * scale).astype(bf),
            "wkT": np.ascontiguousarray(
                np.asarray(Wk, np.float32).T[:, cols]).astype(bf),
            "wvT": np.ascontiguousarray(
                np.asarray(Wv, np.float32).T[:, cols]).astype(bf),
            "woT": np.ascontiguousarray(
                np.asarray(Wo, np.float32).T[cols, :]).astype(bf),
            "bc33": bc,
            "ones33": ones33,
        })
    kw = dict(trace_kwargs or {})
    res = run_bass_kernel_spmd(
        _get_nc(), in_maps, list(range(N_CORES)), trace=trace, **kw
    )
    y = np.empty((B, S, D), np.float32)
    for b in range(B):
        y[b] = res.results[2 * b]["y"] + res.results[2 * b + 1]["y"]
    return y, res


def kernel(x, Wq, Wk, Wv, Wo):
    y, _ = run(x, Wq, Wk, Wv, Wo)
    return y


# revision 21
# speedup vs baseline: 1.0692x; 1.0550x over previous
"""Multi-head causal self-attention on 8 Trainium2 NeuronCores.

Problem: x [4, 2048, 1024], Wq/Wk/Wv/Wo [1024, 1024] (applied as x @ W.T),
16 heads, dk=64, causal softmax, output [4, 2048, 1024], all fp32.

Sharding: 8 cores = 4 batches x 2 head-groups (8 heads each).
Each core computes QKV projections for its 8 heads, streaming causal
attention, and a partial output projection (Wo row-split). The host adds
the two partial outputs per batch element.

Per-core layouts (chosen so NO on-device transposes are needed):
  xT  [1024, 2048]  = x[b].T          (host-transposed)
  wqT [1024, 512]   = (Wq/8).T cols for this head group (1/sqrt(dk) folded)
  wkT [1024, 512], wvT [1024, 512]
  woT [512, 1024]   = Wo[:, cols].T
  QT/KT on chip as [feat, seq] (head pairs stacked on partitions),
  V as [seq, feat] bf16. scoresT tiles [k=128, q=512] per head pair are
  exp'ed on ScalarE into bf16; the causal mask is applied with
  affine_select on the idle GpSimd engine; the softmax denominator is a
  ones-matmul (partition reduction on the PE); 1/l is broadcast across
  partitions with a tiny constant matmul.

Projection s-chunks and attention q-blocks are interleaved in program
order so TensorE (projections) and ScalarE (exp) work concurrently.
"""

import ml_dtypes
import numpy as np

import concourse.bass as bass
import concourse.mybir as mybir
import concourse.tile as tile
from concourse.bass_utils import run_bass_kernel_spmd
from concourse.vector_clock import ScopedClock

F32 = mybir.dt.float32
F32R = mybir.dt.float32r
BF16 = mybir.dt.bfloat16
AF = mybir.ActivationFunctionType
ALU = mybir.AluOpType

B, S, D = 4, 2048, 1024
H = 16
DK = 64
N_CORES = 8
HG = 512          # head-group width (8 heads x 64)


# ---------------------------------------------------------------------------
# This walrus accepts at most 1 sem wait per instruction (2 for
# EventSemaphore). Tile emits more in two places; both are fixed up here by
# moving excess waits onto preceding instructions on the same engine.
# ---------------------------------------------------------------------------
def _split_drain_and_barrier(self, tick_clock, wait_clock):
    nc = self.nc
    probe = nc.sync.nop(nofuse=True, hint="tile_drain_waits")
    wait_clock.add_sem_waits(
        probe.ins, ScopedClock({None: tick_clock.global_clock})
    )
    si = probe.ins.sync_info
    waits = list(si.on_wait) if si is not None else []
    if len(waits) > 1:
        probe.ins.sync_info = mybir.SyncInfo(on_wait=[waits[0]], on_update=[])
        for w in waits[1:]:
            n = nc.sync.nop(nofuse=True, hint="tile_drain_waits")
            n.ins.sync_info = mybir.SyncInfo(on_wait=[w], on_update=[])
    nc.sync.drain()
    nc.all_engine_barrier()
    popped = nc._tile_sem_poison_stack.pop()
    assert popped is self._sem_poison
    nc.clear_and_free_semaphores(list(self.sems.allocated().values()))
    nc.all_engine_barrier()


tile.TileContext._drain_and_barrier = _split_drain_and_barrier

_wsplit_counter = [0]


def _enforce_wait_limits(m):
    for fn in m.functions:
        for bb in fn.blocks:
            out = []
            changed = False
            for inst in bb.instructions:
                si = inst.sync_info
                cap = 2 if isinstance(inst, mybir.InstEventSemaphore) else 1
                if si is not None and len(si.on_wait) > cap:
                    waits = list(si.on_wait)
                    keep, extra = waits[:cap], waits[cap:]
                    for i in range(0, len(extra), 2):
                        _wsplit_counter[0] += 1
                        out.append(mybir.InstEventSemaphore(
                            name=f"I-wsplit-{_wsplit_counter[0]}",
                            engine=inst.engine,
                            ins=[], outs=[],
                            sync_info=mybir.SyncInfo(
                                on_wait=extra[i:i + 2], on_update=[]),
                        ))
                    inst.sync_info = mybir.SyncInfo(
                        on_wait=keep, on_update=list(si.on_update))
                    changed = True
                out.append(inst)
            if changed:
                bb.instructions = out


def build_nc():
    nc = bass.Bass()

    xT = nc.declare_dram_parameter("xT", [D, S], BF16, isOutput=False)
    wqT = nc.declare_dram_parameter("wqT", [D, HG], BF16, isOutput=False)
    wkT = nc.declare_dram_parameter("wkT", [D, HG], BF16, isOutput=False)
    wvT = nc.declare_dram_parameter("wvT", [D, HG], BF16, isOutput=False)
    woT = nc.declare_dram_parameter("woT", [HG, D], BF16, isOutput=False)
    bc33 = nc.declare_dram_parameter("bc33", [33, 128], F32R, isOutput=False)
    ones33 = nc.declare_dram_parameter("ones33", [33, 512], F32R, isOutput=False)
    yout = nc.declare_dram_parameter("y", [S, D], F32, isOutput=True)

    KT8 = D // 128   # contraction tiles for the projections
    NP = 4           # head pairs per core
    NS = S // 128    # seq tiles of 128

    from contextlib import ExitStack

    with tile.TileContext(nc) as tc, ExitStack() as ctx:
        ep = ctx.enter_context
        consts = ep(tc.tile_pool(name="consts", bufs=1))
        qt_pool = ep(tc.tile_pool(name="qt", bufs=1))
        kt_pool = ep(tc.tile_pool(name="kt", bufs=1))
        v_pool = ep(tc.tile_pool(name="v", bufs=1))
        wo_pool = ep(tc.tile_pool(name="wo", bufs=1))
        wq_pool = ep(tc.tile_pool(name="wq", bufs=1))
        wk_pool = ep(tc.tile_pool(name="wk", bufs=1))
        wv_pool = ep(tc.tile_pool(name="wv", bufs=1))
        xt_pool = ep(tc.tile_pool(name="xt", bufs=2))
        exp_pool = ep(tc.tile_pool(name="exp", bufs=3))
        ctxn_pool = ep(tc.tile_pool(name="ctxn", bufs=12))
        rcp_pool = ep(tc.tile_pool(name="rcp", bufs=2))
        ctxraw_pool = ep(tc.tile_pool(name="ctxraw", bufs=2))
        lpack_pool = ep(tc.tile_pool(name="lpack", bufs=2))
        dram_pool = ep(tc.tile_pool(name="ldram", bufs=2, space="DRAM"))
        lraw_pool = ep(tc.tile_pool(name="lraw", bufs=2))
        bcs_pool = ep(tc.tile_pool(name="bcs", bufs=2))
        ybuf_pool = ep(tc.tile_pool(name="ybuf", bufs=2))
        mm_ps = ep(tc.tile_pool(name="mm_ps", bufs=2, space="PSUM"))
        sc_ps = ep(tc.tile_pool(name="sc_ps", bufs=2, space="PSUM"))
        ctx_ps = ep(tc.tile_pool(name="ctx_ps", bufs=1, space="PSUM"))
        l_ps = ep(tc.tile_pool(name="l_ps", bufs=1, space="PSUM"))

        # ---- constants and weights ----------------------------------------
        bc_t = consts.tile([33, 128], F32R, tag="bc")
        nc.sync.dma_start(bc_t[:], bc33[:])
        ones_t = consts.tile([128, 1], BF16, tag="ones")
        nc.gpsimd.memset(ones_t[:], 1.0)

        QT = [qt_pool.tile([128, S], BF16, tag=f"qt{p}", name=f"QT{p}")
              for p in range(NP)]
        KTt = [kt_pool.tile([128, S], BF16, tag=f"kt{p}", name=f"KTt{p}")
               for p in range(NP)]
        V = [v_pool.tile([128, HG], BF16, tag=f"v{s}", name=f"V{s}")
             for s in range(NS)]
        wo_t = []
        for c in range(NP):
            t = wo_pool.tile([128, D], BF16, tag=f"wo{c}")
            nc.sync.dma_start(t[:], woT[c * 128:(c + 1) * 128, :])
            wo_t.append(t)
        wq_t, wk_t, wv_t = [], [], []
        for kt in range(KT8):
            for pool, lst, src in (
                (wq_pool, wq_t, wqT),
                (wk_pool, wk_t, wkT),
                (wv_pool, wv_t, wvT),
            ):
                t = pool.tile([128, HG], BF16, tag=f"w{kt}")
                nc.sync.dma_start(t[:], src[kt * 128:(kt + 1) * 128, :])
                lst.append(t)

        def emit_xt_dmas(st):
            xts = []
            for kt in range(KT8):
                t = xt_pool.tile([128, 512], BF16, tag=f"xt{kt}",
                                 name=f"xt{st}_{kt}")
                nc.sync.dma_start(
                    t[:], xT[kt * 128:(kt + 1) * 128, st * 512:(st + 1) * 512]
                )
                xts.append(t)
            return xts

        def proj_items(st, xts):
            """QKV projection work for chunk st as a flat list of closures,
            one instruction each, so they can be sprinkled between attention
            triples at fine grain."""
            items = []

            def qk_group(ot, w_t, dst, name):
                holder = {}

                def mk_mm(kt):
                    def go():
                        if "ps" not in holder:
                            holder["ps"] = mm_ps.tile(
                                [128, 512], F32, tag="mm", name=name)
                        nc.tensor.matmul(
                            holder["ps"][:],
                            w_t[kt][:, ot * 128:(ot + 1) * 128],
                            xts[kt][:],
                            start=(kt == 0),
                            stop=(kt == KT8 - 1),
                        )
                    return go

                def copy():
                    nc.vector.tensor_copy(
                        dst[ot][:, st * 512:(st + 1) * 512], holder["ps"][:])

                return [mk_mm(kt) for kt in range(KT8)] + [copy]

            def v_group(sub):
                holder = {}

                def mk_mm(kt):
                    def go():
                        if "ps" not in holder:
                            holder["ps"] = mm_ps.tile(
                                [128, 512], F32, tag="mm", name=f"pv{st}{sub}")
                        nc.tensor.matmul(
                            holder["ps"][:],
                            xts[kt][:, sub * 128:(sub + 1) * 128],
                            wv_t[kt][:],
                            start=(kt == 0),
                            stop=(kt == KT8 - 1),
                        )
                    return go

                def copy():
                    nc.vector.tensor_copy(V[st * 4 + sub][:], holder["ps"][:])

                return [mk_mm(kt) for kt in range(KT8)] + [copy]

            for ot in range(NP):
                items.extend(qk_group(ot, wq_t, QT, f"pq{st}{ot}"))
                items.extend(qk_group(ot, wk_t, KTt, f"pk{st}{ot}"))
            for sub in range(4):
                items.extend(v_group(sub))
            return items

        def norm_part1(lraw, label):
            """1/l via all-lane reciprocal (DRAM repack); sync+DVE only, so it
            can be emitted the moment a pair finishes without stalling PE."""
            ls = dram_pool.tile([2, 512], F32, tag="ls", name=f"ls{label}")
            nc.sync.dma_start(ls[0:1, :], lraw[0:1, :])
            nc.sync.dma_start(ls[1:2, :], lraw[32:33, :])
            lpack = lpack_pool.tile([128, 8], F32, tag="lp", name=f"lp{label}")
            nc.sync.dma_start(
                lpack[:, 0:4], ls[0].rearrange("(p f) -> p f", p=128))
            nc.sync.dma_start(
                lpack[:, 4:8], ls[1].rearrange("(p f) -> p f", p=128))
            rpk = lpack_pool.tile([128, 8], F32R, tag="rp", name=f"rp{label}")
            with nc.allow_low_precision("fp32r attention pipeline"):
                nc.vector.reciprocal(rpk[:], lpack[:])
            rs = dram_pool.tile([2, 512], F32R, tag="rs", name=f"rs{label}")
            nc.sync.dma_start(
                rs[0].rearrange("(p f) -> p f", p=128), rpk[:, 0:4])
            nc.sync.dma_start(
                rs[1].rearrange("(p f) -> p f", p=128), rpk[:, 4:8])
            rcp = rcp_pool.tile([33, 512], F32R, tag="rcp", name=f"rcp{label}")
            nc.sync.dma_start(rcp[:], ones33[:])
            nc.sync.dma_start(rcp[0:1, :], rs[0:1, :])
            nc.sync.dma_start(rcp[32:33, :], rs[1:2, :])
            return rcp

        def norm_part2(rcp, ctxraw, label, sink):
            """Broadcast 1/l across partitions (tiny matmul) and scale ctx."""
            bcp = mm_ps.tile([128, 512], F32, tag="mm", name=f"bcp{label}")
            nc.tensor.matmul(bcp[:], bc_t[:], rcp[:], start=True, stop=True)
            bcs = bcs_pool.tile([128, 512], F32, tag="bcs", name=f"bcs{label}")
            nc.vector.tensor_copy(bcs[:], bcp[:])
            cn = ctxn_pool.tile([128, 512], BF16, tag="cn", name=f"cn{label}")
            nc.vector.tensor_mul(cn[:], ctxraw[:], bcs[:])
            sink.append(cn)

        def attention_block(j, fill, carried_norm):
            """Causal attention + partial output projection for q-tile j.
            `fill` is a list of closures (next chunk's projection groups)
            sprinkled into the PE stream to cover exp-wait stalls.
            `carried_norm` is the previous block's unemitted normalize; the
            one left over here is returned for the next block, so the PE
            stream never stalls on a normalize chain at a block boundary."""
            fill = list(fill)
            n_triples = NP * 4 * (j + 1)
            per_triple = -(-len(fill) // n_triples) if fill else 0

            def emit_fill(n):
                for _ in range(n):
                    if not fill:
                        return
                    if fill[0]() is False:
                        return  # head item's inputs not produced yet
                    fill.pop(0)

            def scores(pair, j, i):
                sc = sc_ps.tile([128, 1024], F32, tag="sc",
                                name=f"sc{j}{pair}{i}")
                qa = QT[pair][0:64, j * 512:(j + 1) * 512]
                qb = QT[pair][64:128, j * 512:(j + 1) * 512]
                ka = KTt[pair][0:64, i * 128:(i + 1) * 128]
                kb = KTt[pair][64:128, i * 128:(i + 1) * 128]
                nc.tensor.matmul(
                    sc[:, 0:512], ka, qa,
                    start=True, stop=True, tile_position=(0, 0),
                )
                nc.tensor.matmul(
                    sc[:, 512:1024], kb, qb,
                    start=True, stop=True, tile_position=(64, 0),
                )
                return sc

            ctxn = []
            ni = 4 * (j + 1)

            def emit_ctx_l(ctx_t, lps, et, i):
                first, last = (i == 0), (i == ni - 1)
                va = V[i][:, pair * 128:pair * 128 + 64]
                vb = V[i][:, pair * 128 + 64:pair * 128 + 128]
                nc.tensor.matmul(
                    ctx_t[0:64, :], va, et[:, 0:512],
                    start=first, stop=last, tile_position=(0, 0),
                )
                nc.tensor.matmul(
                    ctx_t[64:128, :], vb, et[:, 512:1024],
                    start=first, stop=last, tile_position=(0, 64),
                )
                nc.tensor.matmul(
                    lps[0:1, :], ones_t[:], et[:, 0:512],
                    start=first, stop=last, tile_position=(0, 0),
                )
                nc.tensor.matmul(
                    lps[32:33, :], ones_t[:], et[:, 512:1024],
                    start=first, stop=last, tile_position=(0, 32),
                )

            pending_norm = carried_norm
            for pair in range(NP):
                ctx_t = ctx_ps.tile([128, 512], F32, tag="ctx",
                                    name=f"ctx{j}{pair}")
                lps = l_ps.tile([33, 512], F32, tag="l", name=f"l{j}{pair}")
                sc = scores(pair, j, 0)
                pending = None
                for i in range(ni):
                    et = exp_pool.tile([128, 1024], BF16, tag="exp",
                                       name=f"et{j}{pair}{i}")
                    nc.scalar.activation(et[:], sc[:], AF.Exp)
                    if i >= 4 * j:
                        # diagonal block: zero the future positions
                        # keep et[kk, h, qq] iff qq - kk - 128*(i-4j) >= 0
                        p = i - 4 * j
                        nc.gpsimd.affine_select(
                            out=et[:], in_=et[:],
                            pattern=[[0, 2], [1, 512]],
                            compare_op=ALU.is_ge,
                            fill=0.0,
                            base=-128 * p,
                            channel_multiplier=-1,
                        )
                    if i + 1 < ni:
                        sc = scores(pair, j, i + 1)
                    if pending is not None:
                        emit_ctx_l(ctx_t, lps, *pending)
                        emit_fill(per_triple)
                    pending = (et, i)
                emit_ctx_l(ctx_t, lps, *pending)
                emit_fill(per_triple)
                # drain ctx/l PSUM banks to SBUF with single DVE copies so the
                # next pair's matmuls aren't gated on the normalize chain
                ctxraw = ctxraw_pool.tile([128, 512], F32, tag="cr",
                                          name=f"cr{j}{pair}")
                nc.vector.tensor_copy(ctxraw[:], ctx_t[:])
                lraw = lraw_pool.tile([33, 512], F32, tag="lr",
                                      name=f"lr{j}{pair}")
                nc.vector.tensor_copy(lraw[:], lps[:])
                if pending_norm is not None:
                    norm_part2(*pending_norm)
                rcp = norm_part1(lraw, f"{j}{pair}")
                pending_norm = (rcp, ctxraw, f"{j}{pair}", ctxn)

            # drain any remaining fill (all inputs exist by block end)
            while fill:
                assert fill[0]() is not False
                fill.pop(0)
            return ctxn, pending_norm

        def outproj_items(j, ctxn):
            """Output projection for q-tile j as fine-grain fill items."""
            items = []

            def group(s4, oh, holder):
                def mk_mm(pair):
                    def go():
                        if len(ctxn) <= pair:
                            return False  # cn not normalized yet
                        if "ps" not in holder:
                            holder["ps"] = mm_ps.tile(
                                [128, 512], F32, tag="mm", name=f"yp{j}{s4}{oh}")
                        nc.tensor.matmul(
                            holder["ps"][:],
                            ctxn[pair][:, s4 * 128:(s4 + 1) * 128],
                            wo_t[pair][:, oh * 512:(oh + 1) * 512],
                            start=(pair == 0),
                            stop=(pair == NP - 1),
                        )
                    return go

                def copy():
                    nc.vector.tensor_copy(
                        holder["yb"][:, oh * 512:(oh + 1) * 512], holder["ps"][:])
                    del holder["ps"]

                return [mk_mm(p) for p in range(NP)] + [copy]

            for s4 in range(4):
                srow = j * 4 + s4
                holder = {}

                def alloc_yb(holder=holder, s4=s4):
                    holder["yb"] = ybuf_pool.tile(
                        [128, D], F32, tag="yb", name=f"yb{j}{s4}")

                items.append(alloc_yb)
                for oh in range(2):
                    items.extend(group(s4, oh, holder))

                def dma_out(holder=holder, srow=srow):
                    nc.sync.dma_start(
                        yout[srow * 128:(srow + 1) * 128, :], holder["yb"][:])

                items.append(dma_out)
            return items

        # chunk 0 projections run alone; attention block j then carries
        # chunk j+1's projections and block j-1's output projection as PE
        # filler for its exp-wait stalls.
        xts0 = emit_xt_dmas(0)
        for item in proj_items(0, xts0):
            item()
        prev_out = []
        carried = None
        for j in range(4):
            fill = list(prev_out)
            if j + 1 < 4:
                xts = emit_xt_dmas(j + 1)
                fill = proj_items(j + 1, xts) + fill
            ctxn, carried = attention_block(j, fill, carried)
            prev_out = outproj_items(j, ctxn)
        if carried is not None:
            norm_part2(*carried)
        for item in prev_out:
            item()

    _enforce_wait_limits(nc.m)
    return nc


def _host_constants():
    bc = np.zeros((33, 128), np.float32)
    bc[0, 0:64] = 1.0
    bc[32, 64:128] = 1.0
    ones33 = np.ones((33, 512), np.float32)
    return bc, ones33


_NC = None


def _get_nc():
    global _NC
    if _NC is None:
        _NC = build_nc()
    return _NC


def run(x, Wq, Wk, Wv, Wo, trace=False, trace_kwargs=None):
    """Returns (y, BassKernelResults)."""
    x = np.asarray(x, np.float32)
    bc, ones33 = _host_constants()
    scale = 1.0 / np.sqrt(DK)
    in_maps = []
    for core in range(N_CORES):
        b, g = core // 2, core % 2
        cols = slice(g * HG, (g + 1) * HG)
        bf = ml_dtypes.bfloat16
        in_maps.append({
            "xT": np.ascontiguousarray(x[b].T).astype(bf),
            "wqT": np.ascontiguousarray(
                np.asarray(Wq, np.float32).T[:, cols] * scale).astype(bf),
            "wkT": np.ascontiguousarray(
                np.asarray(Wk, np.float32).T[:, cols]).astype(bf),
            "wvT": np.ascontiguousarray(
                np.asarray(Wv, np.float32).T[:, cols]).astype(bf),
            "woT": np.ascontiguousarray(
                np.asarray(Wo, np.float32).T[cols, :]).astype(bf),
            "bc33": bc,
            "ones33": ones33,
        })
    kw = dict(trace_kwargs or {})
    res = run_bass_kernel_spmd(
        _get_nc(), in_maps, list(range(N_CORES)), trace=trace, **kw
    )
    y = np.empty((B, S, D), np.float32)
    for b in range(B):
        y[b] = res.results[2 * b]["y"] + res.results[2 * b + 1]["y"]
    return y, res


def kernel(x, Wq, Wk, Wv, Wo):
    y, _ = run(x, Wq, Wk, Wv, Wo)
    return y


# revision 22
# speedup vs baseline: 1.0807x; 1.0108x over previous
"""Multi-head causal self-attention on 8 Trainium2 NeuronCores.

Problem: x [4, 2048, 1024], Wq/Wk/Wv/Wo [1024, 1024] (applied as x @ W.T),
16 heads, dk=64, causal softmax, output [4, 2048, 1024], all fp32.

Sharding: 8 cores = 4 batches x 2 head-groups (8 heads each).
Each core computes QKV projections for its 8 heads, streaming causal
attention, and a partial output projection (Wo row-split). The host adds
the two partial outputs per batch element.

Per-core layouts (chosen so NO on-device transposes are needed):
  xT  [1024, 2048]  = x[b].T          (host-transposed)
  wqT [1024, 512]   = (Wq/8).T cols for this head group (1/sqrt(dk) folded)
  wkT [1024, 512], wvT [1024, 512]
  woT [512, 1024]   = Wo[:, cols].T
  QT/KT on chip as [feat, seq] (head pairs stacked on partitions),
  V as [seq, feat] bf16. scoresT tiles [k=128, q=512] per head pair are
  exp'ed on ScalarE into bf16; the causal mask is applied with
  affine_select on the idle GpSimd engine; the softmax denominator is a
  ones-matmul (partition reduction on the PE); 1/l is broadcast across
  partitions with a tiny constant matmul.

Projection s-chunks and attention q-blocks are interleaved in program
order so TensorE (projections) and ScalarE (exp) work concurrently.
"""

import ml_dtypes
import numpy as np

import concourse.bass as bass
import concourse.mybir as mybir
import concourse.tile as tile
from concourse.bass_utils import run_bass_kernel_spmd
from concourse.vector_clock import ScopedClock

F32 = mybir.dt.float32
F32R = mybir.dt.float32r
BF16 = mybir.dt.bfloat16
AF = mybir.ActivationFunctionType
ALU = mybir.AluOpType

B, S, D = 4, 2048, 1024
H = 16
DK = 64
N_CORES = 8
HG = 512          # head-group width (8 heads x 64)


# ---------------------------------------------------------------------------
# This walrus accepts at most 1 sem wait per instruction (2 for
# EventSemaphore). Tile emits more in two places; both are fixed up here by
# moving excess waits onto preceding instructions on the same engine.
# ---------------------------------------------------------------------------
def _split_drain_and_barrier(self, tick_clock, wait_clock):
    nc = self.nc
    probe = nc.sync.nop(nofuse=True, hint="tile_drain_waits")
    wait_clock.add_sem_waits(
        probe.ins, ScopedClock({None: tick_clock.global_clock})
    )
    si = probe.ins.sync_info
    waits = list(si.on_wait) if si is not None else []
    if len(waits) > 1:
        probe.ins.sync_info = mybir.SyncInfo(on_wait=[waits[0]], on_update=[])
        for w in waits[1:]:
            n = nc.sync.nop(nofuse=True, hint="tile_drain_waits")
            n.ins.sync_info = mybir.SyncInfo(on_wait=[w], on_update=[])
    nc.sync.drain()
    nc.all_engine_barrier()
    popped = nc._tile_sem_poison_stack.pop()
    assert popped is self._sem_poison
    nc.clear_and_free_semaphores(list(self.sems.allocated().values()))
    nc.all_engine_barrier()


tile.TileContext._drain_and_barrier = _split_drain_and_barrier

_wsplit_counter = [0]


def _enforce_wait_limits(m):
    for fn in m.functions:
        for bb in fn.blocks:
            out = []
            changed = False
            for inst in bb.instructions:
                si = inst.sync_info
                cap = 2 if isinstance(inst, mybir.InstEventSemaphore) else 1
                if si is not None and len(si.on_wait) > cap:
                    waits = list(si.on_wait)
                    keep, extra = waits[:cap], waits[cap:]
                    for i in range(0, len(extra), 2):
                        _wsplit_counter[0] += 1
                        out.append(mybir.InstEventSemaphore(
                            name=f"I-wsplit-{_wsplit_counter[0]}",
                            engine=inst.engine,
                            ins=[], outs=[],
                            sync_info=mybir.SyncInfo(
                                on_wait=extra[i:i + 2], on_update=[]),
                        ))
                    inst.sync_info = mybir.SyncInfo(
                        on_wait=keep, on_update=list(si.on_update))
                    changed = True
                out.append(inst)
            if changed:
                bb.instructions = out


def build_nc():
    nc = bass.Bass()

    xT = nc.declare_dram_parameter("xT", [D, S], BF16, isOutput=False)
    wqT = nc.declare_dram_parameter("wqT", [D, HG], BF16, isOutput=False)
    wkT = nc.declare_dram_parameter("wkT", [D, HG], BF16, isOutput=False)
    wvT = nc.declare_dram_parameter("wvT", [D, HG], BF16, isOutput=False)
    woT = nc.declare_dram_parameter("woT", [HG, D], BF16, isOutput=False)
    bc33 = nc.declare_dram_parameter("bc33", [33, 128], F32R, isOutput=False)
    ones33 = nc.declare_dram_parameter("ones33", [33, 512], F32R, isOutput=False)
    yout = nc.declare_dram_parameter("y", [S, D], F32, isOutput=True)

    KT8 = D // 128   # contraction tiles for the projections
    NP = 4           # head pairs per core
    NS = S // 128    # seq tiles of 128

    from contextlib import ExitStack

    with tile.TileContext(nc) as tc, ExitStack() as ctx:
        ep = ctx.enter_context
        consts = ep(tc.tile_pool(name="consts", bufs=1))
        qt_pool = ep(tc.tile_pool(name="qt", bufs=1))
        kt_pool = ep(tc.tile_pool(name="kt", bufs=1))
        v_pool = ep(tc.tile_pool(name="v", bufs=1))
        wo_pool = ep(tc.tile_pool(name="wo", bufs=1))
        wq_pool = ep(tc.tile_pool(name="wq", bufs=1))
        wk_pool = ep(tc.tile_pool(name="wk", bufs=1))
        wv_pool = ep(tc.tile_pool(name="wv", bufs=1))
        xt_pool = ep(tc.tile_pool(name="xt", bufs=2))
        exp_pool = ep(tc.tile_pool(name="exp", bufs=3))
        ctxn_pool = ep(tc.tile_pool(name="ctxn", bufs=12))
        rcp_pool = ep(tc.tile_pool(name="rcp", bufs=2))
        ctxraw_pool = ep(tc.tile_pool(name="ctxraw", bufs=2))
        lpack_pool = ep(tc.tile_pool(name="lpack", bufs=2))
        dram_pool = ep(tc.tile_pool(name="ldram", bufs=2, space="DRAM"))
        lraw_pool = ep(tc.tile_pool(name="lraw", bufs=2))
        bcs_pool = ep(tc.tile_pool(name="bcs", bufs=2))
        ybuf_pool = ep(tc.tile_pool(name="ybuf", bufs=2))
        mm_ps = ep(tc.tile_pool(name="mm_ps", bufs=2, space="PSUM"))
        sc_ps = ep(tc.tile_pool(name="sc_ps", bufs=2, space="PSUM"))
        ctx_ps = ep(tc.tile_pool(name="ctx_ps", bufs=1, space="PSUM"))
        l_ps = ep(tc.tile_pool(name="l_ps", bufs=1, space="PSUM"))

        # ---- constants and weights ----------------------------------------
        bc_t = consts.tile([33, 128], F32R, tag="bc")
        nc.sync.dma_start(bc_t[:], bc33[:])
        ones_t = consts.tile([128, 1], BF16, tag="ones")
        nc.gpsimd.memset(ones_t[:], 1.0)

        QT = [qt_pool.tile([128, S], BF16, tag=f"qt{p}", name=f"QT{p}")
              for p in range(NP)]
        KTt = [kt_pool.tile([128, S], BF16, tag=f"kt{p}", name=f"KTt{p}")
               for p in range(NP)]
        V = [v_pool.tile([128, HG], BF16, tag=f"v{s}", name=f"V{s}")
             for s in range(NS)]
        # DMA order matters for startup latency: the first projection
        # psum-group needs wq + chunk-0 x tiles, so those go first; wo is not
        # needed until the first output projection (~80us in) and goes last.
        wo_t = []
        wq_t, wk_t, wv_t = [], [], []
        for kt in range(KT8):
            for pool, lst, srcp in (
                (wq_pool, wq_t, wqT),
                (wk_pool, wk_t, wkT),
                (wv_pool, wv_t, wvT),
            ):
                t = pool.tile([128, HG], BF16, tag=f"w{kt}")
                nc.sync.dma_start(t[:], srcp[kt * 128:(kt + 1) * 128, :])
                lst.append(t)

        def emit_xt_dmas(st):
            xts = []
            for kt in range(KT8):
                t = xt_pool.tile([128, 512], BF16, tag=f"xt{kt}",
                                 name=f"xt{st}_{kt}")
                nc.sync.dma_start(
                    t[:], xT[kt * 128:(kt + 1) * 128, st * 512:(st + 1) * 512]
                )
                xts.append(t)
            return xts

        def proj_items(st, xts):
            """QKV projection work for chunk st as a flat list of closures,
            one instruction each, so they can be sprinkled between attention
            triples at fine grain."""
            items = []

            def qk_group(ot, w_t, dst, name):
                holder = {}

                def mk_mm(kt):
                    def go():
                        if "ps" not in holder:
                            holder["ps"] = mm_ps.tile(
                                [128, 512], F32, tag="mm", name=name)
                        nc.tensor.matmul(
                            holder["ps"][:],
                            w_t[kt][:, ot * 128:(ot + 1) * 128],
                            xts[kt][:],
                            start=(kt == 0),
                            stop=(kt == KT8 - 1),
                        )
                    return go

                def copy():
                    nc.vector.tensor_copy(
                        dst[ot][:, st * 512:(st + 1) * 512], holder["ps"][:])

                return [mk_mm(kt) for kt in range(KT8)] + [copy]

            def v_group(sub):
                holder = {}

                def mk_mm(kt):
                    def go():
                        if "ps" not in holder:
                            holder["ps"] = mm_ps.tile(
                                [128, 512], F32, tag="mm", name=f"pv{st}{sub}")
                        nc.tensor.matmul(
                            holder["ps"][:],
                            xts[kt][:, sub * 128:(sub + 1) * 128],
                            wv_t[kt][:],
                            start=(kt == 0),
                            stop=(kt == KT8 - 1),
                        )
                    return go

                def copy():
                    nc.vector.tensor_copy(V[st * 4 + sub][:], holder["ps"][:])

                return [mk_mm(kt) for kt in range(KT8)] + [copy]

            for ot in range(NP):
                items.extend(qk_group(ot, wq_t, QT, f"pq{st}{ot}"))
                items.extend(qk_group(ot, wk_t, KTt, f"pk{st}{ot}"))
            for sub in range(4):
                items.extend(v_group(sub))
            return items

        def norm_part1(lraw, label):
            """1/l via all-lane reciprocal (DRAM repack); sync+DVE only, so it
            can be emitted the moment a pair finishes without stalling PE."""
            ls = dram_pool.tile([2, 512], F32, tag="ls", name=f"ls{label}")
            nc.sync.dma_start(ls[0:1, :], lraw[0:1, :])
            nc.sync.dma_start(ls[1:2, :], lraw[32:33, :])
            lpack = lpack_pool.tile([128, 8], F32, tag="lp", name=f"lp{label}")
            nc.sync.dma_start(
                lpack[:, 0:4], ls[0].rearrange("(p f) -> p f", p=128))
            nc.sync.dma_start(
                lpack[:, 4:8], ls[1].rearrange("(p f) -> p f", p=128))
            rpk = lpack_pool.tile([128, 8], F32R, tag="rp", name=f"rp{label}")
            with nc.allow_low_precision("fp32r attention pipeline"):
                nc.vector.reciprocal(rpk[:], lpack[:])
            rs = dram_pool.tile([2, 512], F32R, tag="rs", name=f"rs{label}")
            nc.sync.dma_start(
                rs[0].rearrange("(p f) -> p f", p=128), rpk[:, 0:4])
            nc.sync.dma_start(
                rs[1].rearrange("(p f) -> p f", p=128), rpk[:, 4:8])
            rcp = rcp_pool.tile([33, 512], F32R, tag="rcp", name=f"rcp{label}")
            nc.sync.dma_start(rcp[:], ones33[:])
            nc.sync.dma_start(rcp[0:1, :], rs[0:1, :])
            nc.sync.dma_start(rcp[32:33, :], rs[1:2, :])
            return rcp

        def norm_part2(rcp, ctxraw, label, sink):
            """Broadcast 1/l across partitions (tiny matmul) and scale ctx."""
            bcp = mm_ps.tile([128, 512], F32, tag="mm", name=f"bcp{label}")
            nc.tensor.matmul(bcp[:], bc_t[:], rcp[:], start=True, stop=True)
            bcs = bcs_pool.tile([128, 512], F32, tag="bcs", name=f"bcs{label}")
            nc.vector.tensor_copy(bcs[:], bcp[:])
            cn = ctxn_pool.tile([128, 512], BF16, tag="cn", name=f"cn{label}")
            nc.vector.tensor_mul(cn[:], ctxraw[:], bcs[:])
            sink.append(cn)

        def attention_block(j, fill, carried_norm):
            """Causal attention + partial output projection for q-tile j.
            `fill` is a list of closures (next chunk's projection groups)
            sprinkled into the PE stream to cover exp-wait stalls.
            `carried_norm` is the previous block's unemitted normalize; the
            one left over here is returned for the next block, so the PE
            stream never stalls on a normalize chain at a block boundary."""
            fill = list(fill)
            n_triples = NP * 4 * (j + 1)
            per_triple = -(-len(fill) // n_triples) if fill else 0

            def emit_fill(n):
                for _ in range(n):
                    if not fill:
                        return
                    if fill[0]() is False:
                        return  # head item's inputs not produced yet
                    fill.pop(0)

            def scores(pair, j, i):
                sc = sc_ps.tile([128, 1024], F32, tag="sc",
                                name=f"sc{j}{pair}{i}")
                qa = QT[pair][0:64, j * 512:(j + 1) * 512]
                qb = QT[pair][64:128, j * 512:(j + 1) * 512]
                ka = KTt[pair][0:64, i * 128:(i + 1) * 128]
                kb = KTt[pair][64:128, i * 128:(i + 1) * 128]
                nc.tensor.matmul(
                    sc[:, 0:512], ka, qa,
                    start=True, stop=True, tile_position=(0, 0),
                )
                nc.tensor.matmul(
                    sc[:, 512:1024], kb, qb,
                    start=True, stop=True, tile_position=(64, 0),
                )
                return sc

            ctxn = []
            ni = 4 * (j + 1)

            def emit_ctx_l(ctx_t, lps, et, i):
                first, last = (i == 0), (i == ni - 1)
                va = V[i][:, pair * 128:pair * 128 + 64]
                vb = V[i][:, pair * 128 + 64:pair * 128 + 128]
                nc.tensor.matmul(
                    ctx_t[0:64, :], va, et[:, 0:512],
                    start=first, stop=last, tile_position=(0, 0),
                )
                nc.tensor.matmul(
                    ctx_t[64:128, :], vb, et[:, 512:1024],
                    start=first, stop=last, tile_position=(0, 64),
                )
                nc.tensor.matmul(
                    lps[0:1, :], ones_t[:], et[:, 0:512],
                    start=first, stop=last, tile_position=(0, 0),
                )
                nc.tensor.matmul(
                    lps[32:33, :], ones_t[:], et[:, 512:1024],
                    start=first, stop=last, tile_position=(0, 32),
                )

            pending_norm = carried_norm
            for pair in range(NP):
                ctx_t = ctx_ps.tile([128, 512], F32, tag="ctx",
                                    name=f"ctx{j}{pair}")
                lps = l_ps.tile([33, 512], F32, tag="l", name=f"l{j}{pair}")
                sc = scores(pair, j, 0)
                pending = None
                for i in range(ni):
                    et = exp_pool.tile([128, 1024], BF16, tag="exp",
                                       name=f"et{j}{pair}{i}")
                    nc.scalar.activation(et[:], sc[:], AF.Exp)
                    if i >= 4 * j:
                        # diagonal block: zero the future positions
                        # keep et[kk, h, qq] iff qq - kk - 128*(i-4j) >= 0
                        p = i - 4 * j
                        nc.gpsimd.affine_select(
                            out=et[:], in_=et[:],
                            pattern=[[0, 2], [1, 512]],
                            compare_op=ALU.is_ge,
                            fill=0.0,
                            base=-128 * p,
                            channel_multiplier=-1,
                        )
                    if i + 1 < ni:
                        sc = scores(pair, j, i + 1)
                    if pending is not None:
                        emit_ctx_l(ctx_t, lps, *pending)
                        emit_fill(per_triple)
                    pending = (et, i)
                emit_ctx_l(ctx_t, lps, *pending)
                emit_fill(per_triple)
                # drain ctx/l PSUM banks to SBUF with single DVE copies so the
                # next pair's matmuls aren't gated on the normalize chain
                ctxraw = ctxraw_pool.tile([128, 512], F32, tag="cr",
                                          name=f"cr{j}{pair}")
                nc.vector.tensor_copy(ctxraw[:], ctx_t[:])
                lraw = lraw_pool.tile([33, 512], F32, tag="lr",
                                      name=f"lr{j}{pair}")
                nc.vector.tensor_copy(lraw[:], lps[:])
                if pending_norm is not None:
                    norm_part2(*pending_norm)
                rcp = norm_part1(lraw, f"{j}{pair}")
                pending_norm = (rcp, ctxraw, f"{j}{pair}", ctxn)

            # drain any remaining fill (all inputs exist by block end)
            while fill:
                assert fill[0]() is not False
                fill.pop(0)
            return ctxn, pending_norm

        def outproj_items(j, ctxn):
            """Output projection for q-tile j as fine-grain fill items."""
            items = []

            def group(s4, oh, holder):
                def mk_mm(pair):
                    def go():
                        if len(ctxn) <= pair:
                            return False  # cn not normalized yet
                        if "ps" not in holder:
                            holder["ps"] = mm_ps.tile(
                                [128, 512], F32, tag="mm", name=f"yp{j}{s4}{oh}")
                        nc.tensor.matmul(
                            holder["ps"][:],
                            ctxn[pair][:, s4 * 128:(s4 + 1) * 128],
                            wo_t[pair][:, oh * 512:(oh + 1) * 512],
                            start=(pair == 0),
                            stop=(pair == NP - 1),
                        )
                    return go

                def copy():
                    nc.vector.tensor_copy(
                        holder["yb"][:, oh * 512:(oh + 1) * 512], holder["ps"][:])
                    del holder["ps"]

                return [mk_mm(p) for p in range(NP)] + [copy]

            for s4 in range(4):
                srow = j * 4 + s4
                holder = {}

                def alloc_yb(holder=holder, s4=s4):
                    holder["yb"] = ybuf_pool.tile(
                        [128, D], F32, tag="yb", name=f"yb{j}{s4}")

                items.append(alloc_yb)
                for oh in range(2):
                    items.extend(group(s4, oh, holder))

                def dma_out(holder=holder, srow=srow):
                    nc.sync.dma_start(
                        yout[srow * 128:(srow + 1) * 128, :], holder["yb"][:])

                items.append(dma_out)
            return items

        # chunk 0 projections run alone; attention block j then carries
        # chunk j+1's projections and block j-1's output projection as PE
        # filler for its exp-wait stalls.
        xts0 = emit_xt_dmas(0)
        for item in proj_items(0, xts0):
            item()
        wo_t.clear()
        for c in range(NP):
            t = wo_pool.tile([128, D], BF16, tag=f"wo{c}")
            nc.sync.dma_start(t[:], woT[c * 128:(c + 1) * 128, :])
            wo_t.append(t)
        prev_out = []
        carried = None
        for j in range(4):
            fill = list(prev_out)
            if j + 1 < 4:
                xts = emit_xt_dmas(j + 1)
                fill = proj_items(j + 1, xts) + fill
            ctxn, carried = attention_block(j, fill, carried)
            prev_out = outproj_items(j, ctxn)
        if carried is not None:
            norm_part2(*carried)
        for item in prev_out:
            item()

    _enforce_wait_limits(nc.m)
    return nc


def _host_constants():
    bc = np.zeros((33, 128), np.float32)
    bc[0, 0:64] = 1.0
    bc[32, 64:128] = 1.0
    ones33 = np.ones((33, 512), np.float32)
    return bc, ones33


_NC = None


def _get_nc():
    global _NC
    if _NC is None:
        _NC = build_nc()
    return _NC


def run(x, Wq, Wk, Wv, Wo, trace=False, trace_kwargs=None):
    """Returns (y, BassKernelResults)."""
    x = np.asarray(x, np.float32)
    bc, ones33 = _host_constants()
    scale = 1.0 / np.sqrt(DK)
    in_maps = []
    for core in range(N_CORES):
        b, g = core // 2, core % 2
        cols = slice(g * HG, (g + 1) * HG)
        bf = ml_dtypes.bfloat16
        in_maps.append({
            "xT": np.ascontiguousarray(x[b].T).astype(bf),
            "wqT": np.ascontiguousarray(
                np.asarray(Wq, np.float32).T[:, cols] * scale).astype(bf),
            "wkT": np.ascontiguousarray(
                np.asarray(Wk, np.float32).T[:, cols]).astype(bf),
            "wvT": np.ascontiguousarray(
                np.asarray(Wv, np.float32).T[:, cols]).astype(bf),
            "woT": np.ascontiguousarray(
                np.asarray(Wo, np.float32).T[cols, :]).astype(bf),
            "bc33": bc,
            "ones33": ones33,
        })
    kw = dict(trace_kwargs or {})
    res = run_bass_kernel_spmd(
        _get_nc(), in_maps, list(range(N_CORES)), trace=trace, **kw
    )
    y = np.empty((B, S, D), np.float32)
    for b in range(B):
        y[b] = res.results[2 * b]["y"] + res.results[2 * b + 1]["y"]
    return y, res


def kernel(x, Wq, Wk, Wv, Wo):
    y, _ = run(x, Wq, Wk, Wv, Wo)
    return y
